# revision 1
# baseline (speedup 1.0000x reference)
"""Trainium2 Bass kernel for nn_CrossAttentionWithMask.

Math (per support image n; B=1, C=64, H=W=64, L=4096):
    Q = q @ Wq.T + bq ; K = s @ Wk.T + bk ; V = s @ Wv.T + bv     [L, C]
    S = (Q @ K.T) * C**-0.5                                       [L, L]
    P = softmax(S, axis=-1)
    mask = sigmoid((max_m P - sigmoid(threshold)) * softplus(temperature))
    out = (P @ V) * mask[:, None]   -> reshaped to [C, H, W]

Sharding: 8 cores = (n in 0..3) x (half of the L query rows). Each core
computes a [2048, 4096] attention block fully independently.

Device dataflow (all in transposed [C, L] layout, which is the native
layout of the inputs):
    Ghat = [[Wk.T@Wq, Wk.T@bq], [bk@Wq, bk@bq]] * scale   (65x65, host)
    P65  = Ghat @ qhatT          (qhatT = [qT; ones])      [65, 2048]
    S^T[m, l] = shatT[:, m] . P65[:, l]                    (PE)
    es = exp(S^T)  (no max subtraction needed; |S| < ~6)   (ACT, bf16 out)
    AV: [V | ones].T @ es accumulated over m-chunks -> [65, l]; row 64 is
        the softmax denominator (PE)
    row-max of es via running tensor_max + PE transpose + free-axis reduce
    final scale = sigmoid(maxattn*tmp - thr*tmp) / denom, applied along l.
"""

import numpy as np

C = 64
L = 4096
L2 = 2048  # per-core query columns
CH = 65    # augmented (homogeneous) dim
NM = L // 128   # 32 m-chunks
N_CORES = 8

_CACHE = {}


def _build_bass():
    import concourse.bass as bass
    import concourse.mybir as mybir
    import concourse.tile as tile
    from concourse import bacc
    from concourse.masks import make_identity

    f32 = mybir.dt.float32
    bf16 = mybir.dt.bfloat16
    AF = mybir.ActivationFunctionType
    X = mybir.AxisListType.X

    nc = bacc.Bacc()
    qT = nc.declare_dram_parameter("qT", [CH, L2], bf16, isOutput=False)
    sT = nc.declare_dram_parameter("sT", [CH, L], bf16, isOutput=False)
    GW = nc.declare_dram_parameter("GW", [CH, 2 * CH], bf16, isOutput=False)
    MP = nc.declare_dram_parameter("MP", [1, 2], f32, isOutput=False)
    OUT = nc.declare_dram_parameter("out", [C, L2], f32, isOutput=True)

    with tile.TileContext(nc) as tc:
        with (
            tc.tile_pool(name="consts", bufs=1) as consts,
            tc.tile_pool(name="big", bufs=1) as big,
            tc.tile_pool(name="es_pool", bufs=6) as es_pool,
            tc.tile_pool(name="tail", bufs=1) as tailp,
        ):
            # ---- constants (staged through DVE so matmuls wait on one sem) ----
            gw = consts.tile([CH, 2 * CH], bf16)
            nc.sync.dma_start(out=gw, in_=GW[:, :])
            gt = gw[:, 0:CH]
            wv = gw[:, CH:2 * CH]

            # ---- augmented inputs in SBUF (ones row appended host-side) ----
            qat = big.tile([CH, L2], bf16)
            nc.sync.dma_start(out=qat, in_=qT[:, :])
            sa = []
            for t2 in range(2):
                t = big.tile([CH, L2], bf16, tag=f"sa{t2}", name=f"sa{t2}")
                nc.sync.dma_start(out=t, in_=sT[:, t2 * L2:(t2 + 1) * L2])
                sa.append(t)
            mpc = consts.tile([128, 2], f32)
            nc.sync.dma_start(
                out=mpc,
                in_=bass.AP(tensor=MP, offset=0, ap=[[0, 128], [1, 2]]),
            )
            ident = consts.tile([128, 128], bf16)
            make_identity(nc, ident)
            ones11 = consts.tile([1, 1], f32)
            nc.vector.memset(ones11, 1.0)
            ones64 = consts.tile([1, C], bf16)
            nc.vector.memset(ones64, 1.0)

            pb = [big.tile([CH, 1024], bf16, tag=f"pb{h}", name=f"pb{h}") for h in range(2)]
            vbuf = big.tile([128, NM, CH], bf16)
            rm = big.tile([128, L2], bf16)

            # ---- projections (PSUM pool closed before the main loop) ----
            # Vaug first so the later pb copies subsume vbuf's DVE ticks.
            with tc.tile_pool(name="proj_psum", bufs=2, space="PSUM") as pj:
                # P65 = Ghat @ qhatT (first: pb gates the main loop)
                for h in range(2):
                    for j in range(2):
                        pp = pj.tile([CH, 512], f32, tag="pp")
                        nc.tensor.matmul(
                            pp, gt,
                            qat[:, (h * 2 + j) * 512:(h * 2 + j + 1) * 512],
                            start=True, stop=True,
                        )
                        nc.vector.tensor_copy(pb[h][:, j * 512:(j + 1) * 512], pp)
                # Vaug chunks, batched 7 per PSUM bank
                for base in range(0, NM, 7):
                    cnt = min(7, NM - base)
                    vpb = pj.tile([128, 7, CH], f32, tag="vp", name=f"vpb{base}")
                    for i in range(cnt):
                        m = base + i
                        t2, mc = divmod(m, 16)
                        nc.tensor.matmul(
                            vpb[:, i, :], sa[t2][:, mc * 128:(mc + 1) * 128], wv,
                            start=True, stop=True,
                        )
                    nc.vector.tensor_copy(
                        vbuf[:, base:base + cnt, :], vpb[:, 0:cnt, :]
                    )
            nc.gpsimd.memset(rm, 0.0)

            # ---- main loop ----
            with tc.tile_pool(name="av_psum", bufs=1, space="PSUM") as avp:
                avst = avp.tile([CH, 4, 512], f32, tag="av", name="avst")
                with tc.tile_pool(name="sp_psum", bufs=2, space="PSUM") as spp:
                    for m in range(NM):
                        t2, mc = divmod(m, 16)
                        lhs = sa[t2][:, mc * 128:(mc + 1) * 128]
                        for h in range(2):
                            sp = spp.tile([128, 1024], f32, tag="sp")
                            for j in range(2):
                                nc.tensor.matmul(
                                    sp[:, j * 512:(j + 1) * 512],
                                    lhs,
                                    pb[h][:, j * 512:(j + 1) * 512],
                                    start=True, stop=True,
                                )
                            es = es_pool.tile([128, 1024], bf16, tag="es")
                            nc.scalar.activation(es, sp, AF.Exp)
                            nc.vector.tensor_max(
                                rm[:, h * 1024:(h + 1) * 1024],
                                rm[:, h * 1024:(h + 1) * 1024],
                                es,
                            )
                            for j in range(2):
                                lt = h * 2 + j
                                nc.tensor.matmul(
                                    avst[:, lt, :],
                                    vbuf[:, m, :],
                                    es[:, j * 512:(j + 1) * 512],
                                    start=(m == 0), stop=(m == NM - 1),
                                )

                # ---- tail: row-max, denominator, mask, final scale ----
                # denominator row (PSUM partition 64) -> SBUF via ACT
                drow = tailp.tile([1, L2], f32)
                nc.scalar.copy(
                    drow.rearrange("o (a b) -> o a b", a=4),
                    avst[CH - 1:CH, :, :],
                )
                rx = tailp.tile([128, 16], f32)
                with tc.tile_pool(name="tail_psum", bufs=2, space="PSUM") as tpp:
                    # row-max of rm via PE transpose + free-axis reduce
                    for g in range(2):
                        tp = tpp.tile([128, 8, 128], bf16, tag="tp", name=f"tp{g}")
                        for i in range(8):
                            j = g * 8 + i
                            nc.tensor.transpose(
                                tp[:, i, :], rm[:, j * 128:(j + 1) * 128], ident
                            )
                        nc.vector.reduce_max(rx[:, g * 8:(g + 1) * 8], tp, axis=X)
                    # denom [1, 2048] -> [128, 16] via 16 k=1 matmuls
                    dd = tpp.tile([128, 16], f32, tag="tp", name="dd")
                    for j in range(16):
                        nc.tensor.matmul(
                            dd[:, j:j + 1],
                            drow[0:1, j * 128:(j + 1) * 128],
                            ones11,
                            start=True, stop=True,
                        )
                    rd = tailp.tile([128, 16], f32)
                    nc.vector.reciprocal(rd, dd)
                    maxattn = tailp.tile([128, 16], f32)
                    nc.vector.tensor_mul(maxattn, rx, rd)
                    cmask = tailp.tile([128, 16], f32)
                    nc.scalar.activation(
                        cmask, maxattn, AF.Sigmoid,
                        bias=mpc[:, 1:2], scale=mpc[:, 0:1],
                    )
                    cc = tailp.tile([128, 16], bf16)
                    nc.vector.tensor_mul(cc, cmask, rd)
                    # transpose cc -> [16, 128] and replicate over partitions via PE
                    ccT = tpp.tile([16, 128], bf16, tag="tp", name="ccT")
                    nc.tensor.transpose(ccT, cc, ident)
                    ccT_sb = tailp.tile([16, 128], bf16)
                    nc.scalar.copy(ccT_sb, ccT)
                    ccrow = tailp.tile([1, L2], bf16)
                    nc.sync.dma_start(out=ccrow, in_=ccT_sb)
                    out_sb = tailp.tile([C, L2], f32)
                    for g2 in range(2):
                        crp = tpp.tile([C, 2, 512], f32, tag="tp", name=f"crp{g2}")
                        for i in range(2):
                            nc.tensor.matmul(
                                crp[:, i, :], ones64,
                                ccrow[0:1, (g2 * 2 + i) * 512:(g2 * 2 + i + 1) * 512],
                                start=True, stop=True,
                            )
                        crs = tailp.tile([C, 2, 512], f32, tag=f"crs{g2}",
                                         name=f"crs{g2}")
                        nc.scalar.copy(crs, crp)
                        nc.vector.tensor_mul(
                            out_sb[:, g2 * 1024:(g2 + 1) * 1024],
                            avst[0:C, 2 * g2:2 * g2 + 2, :].rearrange(
                                "c a b -> c (a b)"),
                            crs.rearrange("c a b -> c (a b)"),
                        )
                        nc.sync.dma_start(
                            out=OUT[:, g2 * 1024:(g2 + 1) * 1024],
                            in_=out_sb[:, g2 * 1024:(g2 + 1) * 1024],
                        )

    nc.finalize()
    return nc


def _get_bass():
    if "nc" not in _CACHE:
        _CACHE["nc"] = _build_bass()
    return _CACHE["nc"]


def _host_prep(query, support, Wq, bq, Wk, bk, Wv, bv, threshold, temperature):
    import ml_dtypes
    bf = ml_dtypes.bfloat16
    ones = np.ones((1, L), np.float32)
    q = np.concatenate([np.asarray(query, np.float32).reshape(C, L), ones],
                       axis=0).astype(bf)
    s = np.concatenate(
        [np.asarray(support, np.float32).reshape(4, C, L),
         np.broadcast_to(ones, (4, 1, L))], axis=1).astype(bf)
    s = np.ascontiguousarray(s)
    Wq64 = np.asarray(Wq, np.float64)
    bq64 = np.asarray(bq, np.float64)
    Wk64 = np.asarray(Wk, np.float64)
    bk64 = np.asarray(bk, np.float64)
    Wv64 = np.asarray(Wv, np.float64)
    bv64 = np.asarray(bv, np.float64)
    scale = C ** -0.5

    Ghat = np.zeros((CH, CH), np.float64)
    Ghat[:C, :C] = Wk64.T @ Wq64
    Ghat[C, :C] = bk64 @ Wq64
    Ghat[:C, C] = Wk64.T @ bq64
    Ghat[C, C] = bk64 @ bq64
    Ghat *= scale
    import ml_dtypes as _md
    GT = Ghat.T.astype(_md.bfloat16)

    WvA = np.zeros((CH, CH), np.float64)
    WvA[:C, :C] = Wv64.T
    WvA[C, :C] = bv64
    WvA[C, C] = 1.0
    GW = np.ascontiguousarray(np.concatenate([GT, WvA.astype(_md.bfloat16)], axis=1))

    th = float(np.asarray(threshold, np.float64))
    te = float(np.asarray(temperature, np.float64))
    thr = 1.0 / (1.0 + np.exp(-th))
    tmp = np.log1p(np.exp(-abs(te))) + max(te, 0.0)  # softplus
    MPa = np.array([[tmp, -thr * tmp]], np.float32)

    in_maps = []
    for c in range(N_CORES):
        n, half = divmod(c, 2)
        in_maps.append({
            "qT": np.ascontiguousarray(q[:, half * L2:(half + 1) * L2]),
            "sT": np.ascontiguousarray(s[n]),
            "GW": GW,
            "MP": MPa,
        })
    return in_maps


def kernel(query, support, support_labels, Wq, bq, Wk, bk, Wv, bv,
           threshold, temperature):
    import sys
    if "/opt/trn_rl_repo" not in sys.path:
        try:
            import concourse  # noqa: F401
        except ImportError:
            sys.path.insert(0, "/opt/trn_rl_repo")
    from concourse.bass_utils import run_bass_kernel_spmd

    in_maps = _host_prep(query, support, Wq, bq, Wk, bk, Wv, bv,
                         threshold, temperature)
    nc = _get_bass()
    res = run_bass_kernel_spmd(nc, in_maps, list(range(N_CORES))).results

    out = np.zeros((4, C, L), np.float32)
    for c in range(N_CORES):
        n, half = divmod(c, 2)
        out[n][:, half * L2:(half + 1) * L2] = res[c]["out"]
    return out.reshape(4, C, 64, 64)



# revision 4
# speedup vs baseline: 596.8482x; 596.8482x over previous
"""Trainium2 Bass kernel for nn_CrossAttentionWithMask.

Math (per support image n; B=1, C=64, H=W=64, L=4096):
    Q = q @ Wq.T + bq ; K = s @ Wk.T + bk ; V = s @ Wv.T + bv     [L, C]
    S = (Q @ K.T) * C**-0.5                                       [L, L]
    P = softmax(S, axis=-1)
    mask = sigmoid((max_m P - sigmoid(threshold)) * softplus(temperature))
    out = (P @ V) * mask[:, None]   -> reshaped to [C, H, W]

Sharding: 8 cores = (n in 0..3) x (half of the L query rows). Each core
computes a [2048, 4096] attention block fully independently.

Device dataflow (all in transposed [C, L] layout, which is the native
layout of the inputs):
    Ghat = [[Wk.T@Wq, Wk.T@bq], [bk@Wq, bk@bq]] * scale   (65x65, host)
    P65  = Ghat @ qhatT          (qhatT = [qT; ones])      [65, 2048]
    S^T[m, l] = shatT[:, m] . P65[:, l]                    (PE)
    es = exp(S^T)  (no max subtraction needed; |S| < ~6)   (ACT, bf16 out)
    AV: [V | ones].T @ es accumulated over m-chunks -> [65, l]; row 64 is
        the softmax denominator (PE)
    row-max of es via running tensor_max + PE transpose + free-axis reduce
    final scale = sigmoid(maxattn*tmp - thr*tmp) / denom, applied along l.

The program can be built with reps>1, wrapping the whole body in a
hardware For_i loop; one NEFF execution then runs the body `reps` times
back-to-back. test.py uses two reps variants to measure the true
per-execution HW time as a slope, cancelling dispatch overhead.
"""

import numpy as np

C = 64
L = 4096
L2 = 2048  # per-core query columns
CH = 65    # augmented (homogeneous) dim
NM = L // 128   # 32 m-chunks
N_CORES = 8

_CACHE = {}


def _build_bass(reps=1):
    import concourse.bass as bass
    import concourse.mybir as mybir
    import concourse.tile as tile
    from concourse import bacc
    from concourse.masks import make_identity

    f32 = mybir.dt.float32
    bf16 = mybir.dt.bfloat16
    AF = mybir.ActivationFunctionType
    X = mybir.AxisListType.X

    nc = bacc.Bacc()
    qT = nc.declare_dram_parameter("qT", [CH, L2], bf16, isOutput=False)
    sT = nc.declare_dram_parameter("sT", [CH, L], bf16, isOutput=False)
    GW = nc.declare_dram_parameter("GW", [CH, 2 * CH], bf16, isOutput=False)
    MP = nc.declare_dram_parameter("MP", [1, 2], f32, isOutput=False)
    OUT = nc.declare_dram_parameter("out", [C, L2], f32, isOutput=True)

    def _emit(tc):
        with (
            tc.tile_pool(name="consts", bufs=1) as consts,
            tc.tile_pool(name="big", bufs=1) as big,
            tc.tile_pool(name="es_pool", bufs=6) as es_pool,
            tc.tile_pool(name="tail", bufs=1) as tailp,
        ):
            # ---- constants (staged through DVE so matmuls wait on one sem) ----
            gw = consts.tile([CH, 2 * CH], bf16)
            nc.sync.dma_start(out=gw, in_=GW[:, :])
            gt = gw[:, 0:CH]
            wv = gw[:, CH:2 * CH]

            # ---- augmented inputs in SBUF (ones row appended host-side) ----
            qat = big.tile([CH, L2], bf16)
            nc.sync.dma_start(out=qat, in_=qT[:, :])
            sa = []
            for t2 in range(2):
                t = big.tile([CH, L2], bf16, tag=f"sa{t2}", name=f"sa{t2}")
                nc.sync.dma_start(out=t, in_=sT[:, t2 * L2:(t2 + 1) * L2])
                sa.append(t)
            mpc = consts.tile([128, 2], f32)
            nc.sync.dma_start(
                out=mpc,
                in_=bass.AP(tensor=MP, offset=0, ap=[[0, 128], [1, 2]]),
            )
            ident = consts.tile([128, 128], bf16)
            make_identity(nc, ident)
            ones11 = consts.tile([1, 1], f32)
            nc.vector.memset(ones11, 1.0)
            ones64 = consts.tile([1, C], bf16)
            nc.vector.memset(ones64, 1.0)

            pb = [big.tile([CH, 1024], bf16, tag=f"pb{h}", name=f"pb{h}") for h in range(2)]
            vbuf = big.tile([128, NM, CH], bf16)
            rm = big.tile([128, L2], bf16)

            # ---- projections (PSUM pool closed before the main loop) ----
            # Vaug first so the later pb copies subsume vbuf's DVE ticks.
            with tc.tile_pool(name="proj_psum", bufs=2, space="PSUM") as pj:
                # P65 = Ghat @ qhatT (first: pb gates the main loop)
                for h in range(2):
                    for j in range(2):
                        pp = pj.tile([CH, 512], f32, tag="pp")
                        nc.tensor.matmul(
                            pp, gt,
                            qat[:, (h * 2 + j) * 512:(h * 2 + j + 1) * 512],
                            start=True, stop=True,
                        )
                        nc.vector.tensor_copy(pb[h][:, j * 512:(j + 1) * 512], pp)
                # Vaug chunks, batched 7 per PSUM bank
                for base in range(0, NM, 7):
                    cnt = min(7, NM - base)
                    vpb = pj.tile([128, 7, CH], f32, tag="vp", name=f"vpb{base}")
                    for i in range(cnt):
                        m = base + i
                        t2, mc = divmod(m, 16)
                        nc.tensor.matmul(
                            vpb[:, i, :], sa[t2][:, mc * 128:(mc + 1) * 128], wv,
                            start=True, stop=True,
                        )
                    nc.vector.tensor_copy(
                        vbuf[:, base:base + cnt, :], vpb[:, 0:cnt, :]
                    )
            nc.gpsimd.memset(rm, 0.0)

            # ---- main loop ----
            with tc.tile_pool(name="av_psum", bufs=1, space="PSUM") as avp:
                avst = avp.tile([CH, 4, 512], f32, tag="av", name="avst")
                with tc.tile_pool(name="sp_psum", bufs=2, space="PSUM") as spp:
                    for m in range(NM):
                        t2, mc = divmod(m, 16)
                        lhs = sa[t2][:, mc * 128:(mc + 1) * 128]
                        for h in range(2):
                            sp = spp.tile([128, 1024], f32, tag="sp")
                            for j in range(2):
                                nc.tensor.matmul(
                                    sp[:, j * 512:(j + 1) * 512],
                                    lhs,
                                    pb[h][:, j * 512:(j + 1) * 512],
                                    start=True, stop=True,
                                )
                            es = es_pool.tile([128, 1024], bf16, tag="es")
                            nc.scalar.activation(es, sp, AF.Exp)
                            nc.vector.tensor_max(
                                rm[:, h * 1024:(h + 1) * 1024],
                                rm[:, h * 1024:(h + 1) * 1024],
                                es,
                            )
                            for j in range(2):
                                lt = h * 2 + j
                                nc.tensor.matmul(
                                    avst[:, lt, :],
                                    vbuf[:, m, :],
                                    es[:, j * 512:(j + 1) * 512],
                                    start=(m == 0), stop=(m == NM - 1),
                                )

                # ---- tail: row-max, denominator, mask, final scale ----
                # denominator row (PSUM partition 64) -> SBUF via ACT
                drow = tailp.tile([1, L2], f32)
                nc.scalar.copy(
                    drow.rearrange("o (a b) -> o a b", a=4),
                    avst[CH - 1:CH, :, :],
                )
                rx = tailp.tile([128, 16], f32)
                with tc.tile_pool(name="tail_psum", bufs=2, space="PSUM") as tpp:
                    # row-max of rm via PE transpose + free-axis reduce
                    for g in range(2):
                        tp = tpp.tile([128, 8, 128], bf16, tag="tp", name=f"tp{g}")
                        for i in range(8):
                            j = g * 8 + i
                            nc.tensor.transpose(
                                tp[:, i, :], rm[:, j * 128:(j + 1) * 128], ident
                            )
                        nc.vector.reduce_max(rx[:, g * 8:(g + 1) * 8], tp, axis=X)
                    # denom [1, 2048] -> [128, 16] via 16 k=1 matmuls
                    dd = tpp.tile([128, 16], f32, tag="tp", name="dd")
                    for j in range(16):
                        nc.tensor.matmul(
                            dd[:, j:j + 1],
                            drow[0:1, j * 128:(j + 1) * 128],
                            ones11,
                            start=True, stop=True,
                        )
                    rd = tailp.tile([128, 16], f32)
                    nc.vector.reciprocal(rd, dd)
                    maxattn = tailp.tile([128, 16], f32)
                    nc.vector.tensor_mul(maxattn, rx, rd)
                    cmask = tailp.tile([128, 16], f32)
                    nc.scalar.activation(
                        cmask, maxattn, AF.Sigmoid,
                        bias=mpc[:, 1:2], scale=mpc[:, 0:1],
                    )
                    cc = tailp.tile([128, 16], bf16)
                    nc.vector.tensor_mul(cc, cmask, rd)
                    # transpose cc -> [16, 128] and replicate over partitions via PE
                    ccT = tpp.tile([16, 128], bf16, tag="tp", name="ccT")
                    nc.tensor.transpose(ccT, cc, ident)
                    ccT_sb = tailp.tile([16, 128], bf16)
                    nc.scalar.copy(ccT_sb, ccT)
                    ccrow = tailp.tile([1, L2], bf16)
                    nc.sync.dma_start(out=ccrow, in_=ccT_sb)
                    out_sb = tailp.tile([C, L2], f32)
                    for g2 in range(2):
                        crp = tpp.tile([C, 2, 512], f32, tag="tp", name=f"crp{g2}")
                        for i in range(2):
                            nc.tensor.matmul(
                                crp[:, i, :], ones64,
                                ccrow[0:1, (g2 * 2 + i) * 512:(g2 * 2 + i + 1) * 512],
                                start=True, stop=True,
                            )
                        crs = tailp.tile([C, 2, 512], f32, tag=f"crs{g2}",
                                         name=f"crs{g2}")
                        nc.scalar.copy(crs, crp)
                        nc.vector.tensor_mul(
                            out_sb[:, g2 * 1024:(g2 + 1) * 1024],
                            avst[0:C, 2 * g2:2 * g2 + 2, :].rearrange(
                                "c a b -> c (a b)"),
                            crs.rearrange("c a b -> c (a b)"),
                        )
                        nc.sync.dma_start(
                            out=OUT[:, g2 * 1024:(g2 + 1) * 1024],
                            in_=out_sb[:, g2 * 1024:(g2 + 1) * 1024],
                        )

    with tile.TileContext(nc) as tc:
        if reps == 1:
            _emit(tc)
        else:
            with tc.For_i(0, reps, 1):
                _emit(tc)

    nc.finalize()
    return nc


def _get_bass(reps=1):
    key = ("nc", reps)
    if key not in _CACHE:
        _CACHE[key] = _build_bass(reps)
    return _CACHE[key]


def _introspect(nc):
    import jax
    import concourse.mybir as mybir

    pname = nc.partition_id_tensor.name if nc.partition_id_tensor else None
    in_names, out_names, out_avals, zero_outs = [], [], [], []
    for alloc in nc.m.functions[0].allocations:
        if not isinstance(alloc, mybir.MemoryLocationSet):
            continue
        name = alloc.memorylocations[0].name
        if alloc.kind == "ExternalInput":
            if name != pname:
                in_names.append(name)
        elif alloc.kind == "ExternalOutput":
            shape = tuple(alloc.tensor_shape)
            dtype = mybir.dt.np(alloc.dtype)
            out_names.append(name)
            out_avals.append(jax.core.ShapedArray(shape, dtype))
            zero_outs.append(np.zeros(shape, dtype))
    return pname, in_names, out_names, out_avals, zero_outs


def _get_runner(reps=1):
    """Cached jitted 8-core SPMD dispatcher for the reps-variant program.

    Returns (fn, in_names, out_names, out_avals, zero_outs). fn takes the
    concatenated [8*dim0, ...] host/device arrays (inputs then zero output
    buffers) and returns the concatenated outputs.
    """
    key = ("runner", reps)
    if key in _CACHE:
        return _CACHE[key]

    import jax
    from jax.sharding import Mesh, PartitionSpec
    from jax.experimental.shard_map import shard_map
    from concourse.bass2jax import (
        _bass_exec_p, install_neuronx_cc_hook, partition_id_tensor,
    )

    install_neuronx_cc_hook()
    nc = _get_bass(reps)
    pname, in_names, out_names, out_avals, zero_outs = _introspect(nc)
    n_params = len(in_names)
    all_names = in_names + out_names
    if pname is not None:
        all_names = all_names + [pname]

    def _body(*args):
        operands = list(args)
        if pname is not None:
            operands.append(partition_id_tensor())
        outs = _bass_exec_p.bind(
            *operands,
            out_avals=tuple(out_avals),
            in_names=tuple(all_names),
            out_names=tuple(out_names),
            lowering_input_output_aliases=(),
            sim_require_finite=True,
            sim_require_nnan=True,
            nc=nc,
        )
        return tuple(outs)

    devices = jax.devices()[:N_CORES]
    mesh = Mesh(np.asarray(devices), ("core",))
    nin = n_params + len(out_names)
    fn = jax.jit(shard_map(
        _body, mesh=mesh,
        in_specs=(PartitionSpec("core"),) * nin,
        out_specs=(PartitionSpec("core"),) * len(out_names),
        check_rep=False,
    ), keep_unused=True)
    _CACHE[key] = (fn, in_names, out_names, out_avals, zero_outs)
    return _CACHE[key]


def _host_prep(query, support, Wq, bq, Wk, bk, Wv, bv, threshold, temperature):
    import ml_dtypes
    bf = ml_dtypes.bfloat16
    ones = np.ones((1, L), np.float32)
    q = np.concatenate([np.asarray(query, np.float32).reshape(C, L), ones],
                       axis=0).astype(bf)
    s = np.concatenate(
        [np.asarray(support, np.float32).reshape(4, C, L),
         np.broadcast_to(ones, (4, 1, L))], axis=1).astype(bf)
    s = np.ascontiguousarray(s)
    Wq64 = np.asarray(Wq, np.float64)
    bq64 = np.asarray(bq, np.float64)
    Wk64 = np.asarray(Wk, np.float64)
    bk64 = np.asarray(bk, np.float64)
    Wv64 = np.asarray(Wv, np.float64)
    bv64 = np.asarray(bv, np.float64)
    scale = C ** -0.5

    Ghat = np.zeros((CH, CH), np.float64)
    Ghat[:C, :C] = Wk64.T @ Wq64
    Ghat[C, :C] = bk64 @ Wq64
    Ghat[:C, C] = Wk64.T @ bq64
    Ghat[C, C] = bk64 @ bq64
    Ghat *= scale
    GT = Ghat.T.astype(bf)

    WvA = np.zeros((CH, CH), np.float64)
    WvA[:C, :C] = Wv64.T
    WvA[C, :C] = bv64
    WvA[C, C] = 1.0
    GW = np.ascontiguousarray(np.concatenate([GT, WvA.astype(bf)], axis=1))

    th = float(np.asarray(threshold, np.float64))
    te = float(np.asarray(temperature, np.float64))
    thr = 1.0 / (1.0 + np.exp(-th))
    tmp = np.log1p(np.exp(-abs(te))) + max(te, 0.0)  # softplus
    MPa = np.array([[tmp, -thr * tmp]], np.float32)

    in_maps = []
    for c in range(N_CORES):
        n, half = divmod(c, 2)
        in_maps.append({
            "qT": np.ascontiguousarray(q[:, half * L2:(half + 1) * L2]),
            "sT": np.ascontiguousarray(s[n]),
            "GW": GW,
            "MP": MPa,
        })
    return in_maps


def _concat_args(in_maps, in_names, zero_outs):
    per_core = [[np.asarray(m[nm]) for nm in in_names] for m in in_maps]
    concat_in = [np.concatenate([per_core[c][i] for c in range(N_CORES)], axis=0)
                 for i in range(len(in_names))]
    concat_zeros = [np.zeros((N_CORES * z.shape[0], *z.shape[1:]), z.dtype)
                    for z in zero_outs]
    return concat_in + concat_zeros


def _gather(res_per_core):
    out = np.zeros((4, C, L), np.float32)
    for c in range(N_CORES):
        n, half = divmod(c, 2)
        out[n][:, half * L2:(half + 1) * L2] = res_per_core[c]
    return out.reshape(4, C, 64, 64)


def kernel(query, support, support_labels, Wq, bq, Wk, bk, Wv, bv,
           threshold, temperature):
    import sys
    if "/opt/trn_rl_repo" not in sys.path:
        try:
            import concourse  # noqa: F401
        except ImportError:
            sys.path.insert(0, "/opt/trn_rl_repo")

    in_maps = _host_prep(query, support, Wq, bq, Wk, bk, Wv, bv,
                         threshold, temperature)

    if not _CACHE.get("warm"):
        # First call: route through the standard SPMD helper (compiles the
        # NEFF into the on-disk cache) and build the cached fast runner.
        from concourse.bass_utils import run_bass_kernel_spmd
        nc = _get_bass(1)
        res = run_bass_kernel_spmd(nc, in_maps, list(range(N_CORES))).results
        _get_runner(1)
        _CACHE["warm"] = True
        return _gather([res[c]["out"] for c in range(N_CORES)])

    import jax
    fn, in_names, out_names, out_avals, zero_outs = _get_runner(1)
    args = _concat_args(in_maps, in_names, zero_outs)
    out_arrs = fn(*args)
    res = np.asarray(out_arrs[0]).reshape(N_CORES, *out_avals[0].shape)
    return _gather([res[c] for c in range(N_CORES)])


# revision 13
# speedup vs baseline: 635.0511x; 1.0640x over previous
"""Trainium2 Bass kernel for nn_CrossAttentionWithMask.

Math (per support image n; B=1, C=64, H=W=64, L=4096):
    Q = q @ Wq.T + bq ; K = s @ Wk.T + bk ; V = s @ Wv.T + bv     [L, C]
    S = (Q @ K.T) * C**-0.5                                       [L, L]
    P = softmax(S, axis=-1)
    mask = sigmoid((max_m P - sigmoid(threshold)) * softplus(temperature))
    out = (P @ V) * mask[:, None]   -> reshaped to [C, H, W]

Sharding: 8 cores = (n in 0..3) x (half of the L query rows). Each core
computes a [2048, 4096] attention block fully independently.

Device dataflow (all in transposed [C, L] layout, which is the native
layout of the inputs):
    Ghat = [[Wk.T@Wq, Wk.T@bq], [bk@Wq, bk@bq]] * scale   (65x65, host)
    P65  = Ghat @ qhatT          (qhatT = [qT; ones])      [65, 2048]
    S^T[m, l] = shatT[:, m] . P65[:, l]                    (PE)
    es = exp(S^T)  (no max subtraction needed; |S| < ~6)   (ACT, bf16 out)
    AV: [V | ones].T @ es accumulated over m-chunks -> [65, l]; row 64 is
        the softmax denominator (PE)
    row-max of es via running tensor_max + PE transpose + free-axis reduce
    final scale = sigmoid(maxattn*tmp - thr*tmp) / denom, applied along l.

The program can be built with reps>1, wrapping the whole body in a
hardware For_i loop; one NEFF execution then runs the body `reps` times
back-to-back. test.py uses two reps variants to measure the true
per-execution HW time as a slope, cancelling dispatch overhead.
"""

import numpy as np

C = 64
L = 4096
L2 = 2048  # per-core query columns
CH = 65    # augmented (homogeneous) dim
NM = L // 128   # 32 m-chunks
N_CORES = 8

_CACHE = {}


def _build_bass(reps=1):
    import concourse.bass as bass
    import concourse.mybir as mybir
    import concourse.tile as tile
    from concourse import bacc
    from concourse.masks import make_identity

    f32 = mybir.dt.float32
    bf16 = mybir.dt.bfloat16
    AF = mybir.ActivationFunctionType
    X = mybir.AxisListType.X

    nc = bacc.Bacc()
    qT = nc.declare_dram_parameter("qT", [CH, L2], bf16, isOutput=False)
    sT = nc.declare_dram_parameter("sT", [CH, L], bf16, isOutput=False)
    GW = nc.declare_dram_parameter("GW", [CH, 2 * CH], bf16, isOutput=False)
    MP = nc.declare_dram_parameter("MP", [1, 2], f32, isOutput=False)
    OUT = nc.declare_dram_parameter("out", [C, L2], f32, isOutput=True)

    def _emit(tc):
        with (
            tc.tile_pool(name="consts", bufs=1) as consts,
            tc.tile_pool(name="big", bufs=1) as big,
            tc.tile_pool(name="es_pool", bufs=6) as es_pool,
            tc.tile_pool(name="tail", bufs=1) as tailp,
        ):
            # ---- constants (staged through DVE so matmuls wait on one sem) ----
            gw = consts.tile([CH, 2 * CH], bf16)
            nc.sync.dma_start(out=gw, in_=GW[:, :])
            gt = gw[:, 0:CH]
            wv = gw[:, CH:2 * CH]

            # ---- augmented inputs in SBUF (ones row appended host-side) ----
            qat = big.tile([CH, L2], bf16)
            nc.sync.dma_start(out=qat, in_=qT[:, :])
            sa = []
            for t2 in range(2):
                t = big.tile([CH, L2], bf16, tag=f"sa{t2}", name=f"sa{t2}")
                nc.sync.dma_start(out=t, in_=sT[:, t2 * L2:(t2 + 1) * L2])
                sa.append(t)
            mpc = consts.tile([128, 2], f32)
            nc.sync.dma_start(
                out=mpc,
                in_=bass.AP(tensor=MP, offset=0, ap=[[0, 128], [1, 2]]),
            )
            ident = consts.tile([128, 128], bf16)
            make_identity(nc, ident)
            ones11 = consts.tile([1, 1], f32)
            nc.vector.memset(ones11, 1.0)
            ones64 = consts.tile([1, C], bf16)
            nc.vector.memset(ones64, 1.0)

            pb = [big.tile([CH, 1024], bf16, tag=f"pb{h}", name=f"pb{h}") for h in range(2)]
            # one tile per 7-chunk batch so AV matmuls of early m-chunks
            # don't wait for the full V projection
            vbufs = [big.tile([128, min(7, NM - base), CH], bf16,
                              tag=f"vb{base}", name=f"vb{base}")
                     for base in range(0, NM, 7)]
            rm = big.tile([128, L2], bf16)

            # ---- projections (PSUM pool closed before the main loop) ----
            # Vaug first so the later pb copies subsume vbuf's DVE ticks.
            with tc.tile_pool(name="proj_psum", bufs=2, space="PSUM") as pj:
                # P65 = Ghat @ qhatT (first: pb gates the main loop)
                for h in range(2):
                    for j in range(2):
                        pp = pj.tile([CH, 512], f32, tag="pp")
                        nc.tensor.matmul(
                            pp, gt,
                            qat[:, (h * 2 + j) * 512:(h * 2 + j + 1) * 512],
                            start=True, stop=True,
                        )
                        nc.vector.tensor_copy(pb[h][:, j * 512:(j + 1) * 512], pp)
                # Vaug chunks, batched 7 per PSUM bank
                for bi, base in enumerate(range(0, NM, 7)):
                    cnt = min(7, NM - base)
                    vpb = pj.tile([128, 7, CH], f32, tag="vp", name=f"vpb{base}")
                    for i in range(cnt):
                        m = base + i
                        t2, mc = divmod(m, 16)
                        nc.tensor.matmul(
                            vpb[:, i, :], sa[t2][:, mc * 128:(mc + 1) * 128], wv,
                            start=True, stop=True,
                        )
                    nc.vector.tensor_copy(vbufs[bi], vpb[:, 0:cnt, :])
            nc.gpsimd.memset(rm, 0.0)

            # ---- main loop ----
            with tc.tile_pool(name="av_psum", bufs=1, space="PSUM") as avp:
                avst = avp.tile([CH, 4, 512], f32, tag="av", name="avst")
                with tc.tile_pool(name="sp_psum", bufs=2, space="PSUM") as spp:
                    for m in range(NM):
                        t2, mc = divmod(m, 16)
                        lhs = sa[t2][:, mc * 128:(mc + 1) * 128]
                        for h in range(2):
                            sp = spp.tile([128, 1024], f32, tag="sp")
                            for j in range(2):
                                nc.tensor.matmul(
                                    sp[:, j * 512:(j + 1) * 512],
                                    lhs,
                                    pb[h][:, j * 512:(j + 1) * 512],
                                    start=True, stop=True,
                                )
                            es = es_pool.tile([128, 1024], bf16, tag="es")
                            nc.scalar.activation(es, sp, AF.Exp)
                            nc.vector.tensor_max(
                                rm[:, h * 1024:(h + 1) * 1024],
                                rm[:, h * 1024:(h + 1) * 1024],
                                es,
                            )
                            for j in range(2):
                                lt = h * 2 + j
                                nc.tensor.matmul(
                                    avst[:, lt, :],
                                    vbufs[m // 7][:, m % 7, :],
                                    es[:, j * 512:(j + 1) * 512],
                                    start=(m == 0), stop=(m == NM - 1),
                                )

                # ---- tail: row-max, denominator, mask, final scale ----
                # denominator row (PSUM partition 64) -> SBUF via ACT
                drow = tailp.tile([1, L2], f32)
                nc.scalar.copy(
                    drow.rearrange("o (a b) -> o a b", a=4),
                    avst[CH - 1:CH, :, :],
                )
                rx = tailp.tile([128, 16], f32)
                with tc.tile_pool(name="tail_psum", bufs=2, space="PSUM") as tpp:
                    # row-max of rm via PE transpose + free-axis reduce
                    for g in range(2):
                        tp = tpp.tile([128, 8, 128], bf16, tag="tp", name=f"tp{g}")
                        for i in range(8):
                            j = g * 8 + i
                            nc.tensor.transpose(
                                tp[:, i, :], rm[:, j * 128:(j + 1) * 128], ident
                            )
                        nc.vector.reduce_max(rx[:, g * 8:(g + 1) * 8], tp, axis=X)
                    # denom [1, 2048] -> [128, 16] via 16 k=1 matmuls
                    dd = tpp.tile([128, 16], f32, tag="tp", name="dd")
                    for j in range(16):
                        nc.tensor.matmul(
                            dd[:, j:j + 1],
                            drow[0:1, j * 128:(j + 1) * 128],
                            ones11,
                            start=True, stop=True,
                        )
                    rd = tailp.tile([128, 16], f32)
                    nc.vector.reciprocal(rd, dd)
                    maxattn = tailp.tile([128, 16], f32)
                    nc.vector.tensor_mul(maxattn, rx, rd)
                    cmask = tailp.tile([128, 16], f32)
                    nc.scalar.activation(
                        cmask, maxattn, AF.Sigmoid,
                        bias=mpc[:, 1:2], scale=mpc[:, 0:1],
                    )
                    cc = tailp.tile([128, 16], bf16)
                    nc.vector.tensor_mul(cc, cmask, rd)
                    # transpose cc -> [16, 128] and replicate over partitions via PE
                    ccT = tpp.tile([16, 128], bf16, tag="tp", name="ccT")
                    nc.tensor.transpose(ccT, cc, ident)
                    ccT_sb = tailp.tile([16, 128], bf16)
                    nc.scalar.copy(ccT_sb, ccT)
                    ccrow = tailp.tile([1, L2], bf16)
                    nc.sync.dma_start(out=ccrow, in_=ccT_sb)
                    out_sb = tailp.tile([C, L2], f32)
                    for g2 in range(2):
                        crp = tpp.tile([C, 2, 512], f32, tag="tp", name=f"crp{g2}")
                        for i in range(2):
                            nc.tensor.matmul(
                                crp[:, i, :], ones64,
                                ccrow[0:1, (g2 * 2 + i) * 512:(g2 * 2 + i + 1) * 512],
                                start=True, stop=True,
                            )
                        crs = tailp.tile([C, 2, 512], f32, tag=f"crs{g2}",
                                         name=f"crs{g2}")
                        nc.scalar.copy(crs, crp)
                        nc.vector.tensor_mul(
                            out_sb[:, g2 * 1024:(g2 + 1) * 1024],
                            avst[0:C, 2 * g2:2 * g2 + 2, :].rearrange(
                                "c a b -> c (a b)"),
                            crs.rearrange("c a b -> c (a b)"),
                        )
                        nc.sync.dma_start(
                            out=OUT[:, g2 * 1024:(g2 + 1) * 1024],
                            in_=out_sb[:, g2 * 1024:(g2 + 1) * 1024],
                        )

    with tile.TileContext(nc) as tc:
        if reps == 1:
            _emit(tc)
        else:
            with tc.For_i(0, reps, 1):
                _emit(tc)

    nc.finalize()
    return nc


def _get_bass(reps=1):
    key = ("nc", reps)
    if key not in _CACHE:
        _CACHE[key] = _build_bass(reps)
    return _CACHE[key]


def _introspect(nc):
    import jax
    import concourse.mybir as mybir

    pname = nc.partition_id_tensor.name if nc.partition_id_tensor else None
    in_names, out_names, out_avals, zero_outs = [], [], [], []
    for alloc in nc.m.functions[0].allocations:
        if not isinstance(alloc, mybir.MemoryLocationSet):
            continue
        name = alloc.memorylocations[0].name
        if alloc.kind == "ExternalInput":
            if name != pname:
                in_names.append(name)
        elif alloc.kind == "ExternalOutput":
            shape = tuple(alloc.tensor_shape)
            dtype = mybir.dt.np(alloc.dtype)
            out_names.append(name)
            out_avals.append(jax.core.ShapedArray(shape, dtype))
            zero_outs.append(np.zeros(shape, dtype))
    return pname, in_names, out_names, out_avals, zero_outs


def _get_runner(reps=1):
    """Cached jitted 8-core SPMD dispatcher for the reps-variant program.

    Returns (fn, in_names, out_names, out_avals, zero_outs). fn takes the
    concatenated [8*dim0, ...] host/device arrays (inputs then zero output
    buffers) and returns the concatenated outputs.
    """
    key = ("runner", reps)
    if key in _CACHE:
        return _CACHE[key]

    import jax
    from jax.sharding import Mesh, PartitionSpec
    from jax.experimental.shard_map import shard_map
    from concourse.bass2jax import (
        _bass_exec_p, install_neuronx_cc_hook, partition_id_tensor,
    )

    install_neuronx_cc_hook()
    nc = _get_bass(reps)
    pname, in_names, out_names, out_avals, zero_outs = _introspect(nc)
    n_params = len(in_names)
    all_names = in_names + out_names
    if pname is not None:
        all_names = all_names + [pname]

    def _body(*args):
        operands = list(args)
        if pname is not None:
            operands.append(partition_id_tensor())
        outs = _bass_exec_p.bind(
            *operands,
            out_avals=tuple(out_avals),
            in_names=tuple(all_names),
            out_names=tuple(out_names),
            lowering_input_output_aliases=(),
            sim_require_finite=True,
            sim_require_nnan=True,
            nc=nc,
        )
        return tuple(outs)

    devices = jax.devices()[:N_CORES]
    mesh = Mesh(np.asarray(devices), ("core",))
    nin = n_params + len(out_names)
    fn = jax.jit(shard_map(
        _body, mesh=mesh,
        in_specs=(PartitionSpec("core"),) * nin,
        out_specs=(PartitionSpec("core"),) * len(out_names),
        check_rep=False,
    ), keep_unused=True)
    _CACHE[key] = (fn, in_names, out_names, out_avals, zero_outs)
    return _CACHE[key]


def _host_prep(query, support, Wq, bq, Wk, bk, Wv, bv, threshold, temperature):
    import ml_dtypes
    bf = ml_dtypes.bfloat16
    ones = np.ones((1, L), np.float32)
    q = np.concatenate([np.asarray(query, np.float32).reshape(C, L), ones],
                       axis=0).astype(bf)
    s = np.concatenate(
        [np.asarray(support, np.float32).reshape(4, C, L),
         np.broadcast_to(ones, (4, 1, L))], axis=1).astype(bf)
    s = np.ascontiguousarray(s)
    Wq64 = np.asarray(Wq, np.float64)
    bq64 = np.asarray(bq, np.float64)
    Wk64 = np.asarray(Wk, np.float64)
    bk64 = np.asarray(bk, np.float64)
    Wv64 = np.asarray(Wv, np.float64)
    bv64 = np.asarray(bv, np.float64)
    scale = C ** -0.5

    Ghat = np.zeros((CH, CH), np.float64)
    Ghat[:C, :C] = Wk64.T @ Wq64
    Ghat[C, :C] = bk64 @ Wq64
    Ghat[:C, C] = Wk64.T @ bq64
    Ghat[C, C] = bk64 @ bq64
    Ghat *= scale
    GT = Ghat.T.astype(bf)

    WvA = np.zeros((CH, CH), np.float64)
    WvA[:C, :C] = Wv64.T
    WvA[C, :C] = bv64
    WvA[C, C] = 1.0
    GW = np.ascontiguousarray(np.concatenate([GT, WvA.astype(bf)], axis=1))

    th = float(np.asarray(threshold, np.float64))
    te = float(np.asarray(temperature, np.float64))
    thr = 1.0 / (1.0 + np.exp(-th))
    tmp = np.log1p(np.exp(-abs(te))) + max(te, 0.0)  # softplus
    MPa = np.array([[tmp, -thr * tmp]], np.float32)

    in_maps = []
    for c in range(N_CORES):
        n, half = divmod(c, 2)
        in_maps.append({
            "qT": np.ascontiguousarray(q[:, half * L2:(half + 1) * L2]),
            "sT": np.ascontiguousarray(s[n]),
            "GW": GW,
            "MP": MPa,
        })
    return in_maps


def _concat_args(in_maps, in_names, zero_outs):
    per_core = [[np.asarray(m[nm]) for nm in in_names] for m in in_maps]
    concat_in = [np.concatenate([per_core[c][i] for c in range(N_CORES)], axis=0)
                 for i in range(len(in_names))]
    concat_zeros = [np.zeros((N_CORES * z.shape[0], *z.shape[1:]), z.dtype)
                    for z in zero_outs]
    return concat_in + concat_zeros


def _gather(res_per_core):
    out = np.zeros((4, C, L), np.float32)
    for c in range(N_CORES):
        n, half = divmod(c, 2)
        out[n][:, half * L2:(half + 1) * L2] = res_per_core[c]
    return out.reshape(4, C, 64, 64)


def kernel(query, support, support_labels, Wq, bq, Wk, bk, Wv, bv,
           threshold, temperature):
    import sys
    if "/opt/trn_rl_repo" not in sys.path:
        try:
            import concourse  # noqa: F401
        except ImportError:
            sys.path.insert(0, "/opt/trn_rl_repo")

    in_maps = _host_prep(query, support, Wq, bq, Wk, bk, Wv, bv,
                         threshold, temperature)

    if not _CACHE.get("warm"):
        # First call: route through the standard SPMD helper (compiles the
        # NEFF into the on-disk cache) and build the cached fast runner.
        from concourse.bass_utils import run_bass_kernel_spmd
        nc = _get_bass(1)
        res = run_bass_kernel_spmd(nc, in_maps, list(range(N_CORES))).results
        _get_runner(1)
        _CACHE["warm"] = True
        return _gather([res[c]["out"] for c in range(N_CORES)])

    import jax
    fn, in_names, out_names, out_avals, zero_outs = _get_runner(1)
    args = _concat_args(in_maps, in_names, zero_outs)
    out_arrs = fn(*args)
    res = np.asarray(out_arrs[0]).reshape(N_CORES, *out_avals[0].shape)
    return _gather([res[c] for c in range(N_CORES)])
